# revision 11
# baseline (speedup 1.0000x reference)
"""Bass/Tile kernel for nn_CorrOptL2SDGN, v2: bf16 + merged-halves layout.

Per sequence (Gram reformulation, 4 seqs/core, 8 cores data-parallel):
  M = X X^T (+reg, numerically negligible at 1e-4 vs diag ~484: dropped)
  g_0 = M f_0 + X(-T)
  iter i: h = M g ; num = step*colsum(g^2) ; nden = -colsum(g*h)
          a = num/nden = -step*alpha          (batched divide across seqs)
          [f||g] += a*[g||h]                  (one fused bf16 add)
Layouts (bf16 unless noted):
  S[s]    [128,1936] = [f_h0 | f_h1 | g_h0 | g_h1], c-half h: c = 128h+p
  Mst[s]  [128,2,2,128]: [:,i,j,:] = M[128i+p, 128j+c]  (stationary blocks)
  fst[s]  [121,4,2,128]: [p,h,j,c] = X[128j+c, hw=121h+p]
  negTst  [121,4,484]  : [p,h,f]   = -T[121h+p, f]   (shared)
Engines: PE 8mm/iter, ACT {evict,square}, Pool {td, alpha bcast}, DVE
{u,v muls, fused S-add, divide}.
"""

import sys

sys.path.insert(0, "/opt/trn_rl_repo")

from contextlib import ExitStack

import numpy as np

S_TOTAL, C, F = 32, 256, 484
NCORES = 8
SPC = S_TOTAL // NCORES
NUM_ITER = 5
H = 968
H2 = 1936
H3 = 2904


def build(spc=SPC, num_iter=NUM_ITER):
    import concourse.bacc as bacc
    import concourse.mybir as mybir
    import concourse.tile as tile

    F32 = mybir.dt.float32
    FP16 = mybir.dt.float16
    AF = mybir.ActivationFunctionType
    ALU = mybir.AluOpType

    nc = bacc.Bacc("TRN2", target_bir_lowering=False, debug=False)
    fst_d = nc.dram_tensor("fst", [spc, 121, 4, 2, 128], FP16, kind="ExternalInput")
    ft0_d = nc.dram_tensor("ft0", [spc, 128, H], FP16, kind="ExternalInput")
    gy_d = nc.dram_tensor("gy", [121, 4, 22], FP16, kind="ExternalInput")
    gx_d = nc.dram_tensor("gx", [121, 4, 22], FP16, kind="ExternalInput")
    lsl_d = nc.dram_tensor("lsl", [1], F32, kind="ExternalInput")
    out_d = nc.dram_tensor("outT", [spc, 128, H], F32, kind="ExternalOutput")

    with ExitStack() as ctx:
        tc = ctx.enter_context(tile.TileContext(nc))
        const = ctx.enter_context(tc.tile_pool(name="const", bufs=1))
        state = ctx.enter_context(tc.tile_pool(name="state", bufs=1))
        work = ctx.enter_context(tc.tile_pool(name="work", bufs=2 * spc))
        alph = ctx.enter_context(tc.tile_pool(name="alph", bufs=4))
        psmm = ctx.enter_context(tc.tile_pool(name="psmm", bufs=3, space="PSUM"))
        psnd = ctx.enter_context(tc.tile_pool(name="psnd", bufs=1, space="PSUM"))

        def act_recip(out_ap, in_ap):
            eng = nc.scalar
            ins = [eng.lower_ap(in_ap),
                   mybir.ImmediateValue(dtype=mybir.dt.float32, value=0.0),
                   mybir.ImmediateValue(dtype=mybir.dt.float32, value=1.0),
                   mybir.ImmediateValue(dtype=mybir.dt.float32, value=0.0)]
            eng.add_instruction(mybir.InstActivation(
                name=eng.bass.get_next_instruction_name(),
                func=AF.Reciprocal, ins=ins, outs=[eng.lower_ap(out_ap)]))

        # ---- constants ----
        onesp = const.tile([128, 128], FP16, tag="onesp")
        nc.gpsimd.memset(onesp[:], 1.0)
        step_sb = const.tile([128, 1], F32, tag="step_sb")
        nc.sync.dma_start(step_sb[:], lsl_d.ap().to_broadcast((128, 1)))
        nc.scalar.activation(step_sb[:], step_sb[:], AF.Exp, scale=1.0)
        onesps = const.tile([128, 128], FP16, tag="onesps")
        nc.scalar.activation(onesps[:], onesp[:], AF.Copy, scale=step_sb[:])
        onesn = const.tile([128, 128], FP16, tag="onesn")
        nc.gpsimd.memset(onesn[:], -1.0)
        # sqrt(step): ACT Square scale -> step*g^2
        sqs = const.tile([128, 1], F32, tag="sqs")
        nc.sync.dma_start(sqs[:], lsl_d.ap().to_broadcast((128, 1)))
        nc.scalar.activation(sqs[:], sqs[:], AF.Exp, scale=0.5)

        S, Mst, fstt = {}, {}, {}
        gy = const.tile([121, 4, 22], FP16, tag="gy")
        nc.sync.dma_start(gy[:], gy_d.ap())
        gx = const.tile([121, 4, 22], FP16, tag="gx")
        nc.sync.dma_start(gx[:], gx_d.ap())
        negT = []
        for h in range(4):
            nt = const.tile([121, F], FP16, tag=f"negT{h}")
            nc.vector.tensor_mul(
                nt[:].rearrange("p (a b) -> p a b", a=22),
                gy[:, h, :].unsqueeze(2).broadcast_to((121, 22, 22)),
                gx[:, h, :].unsqueeze(1).broadcast_to((121, 22, 22)))
            negT.append(nt)
        f0t = {}
        for s in range(spc):
            ft = state.tile([121, 4, 2, 128], FP16, tag=f"fst{s}")
            nc.sync.dma_start(ft[:], fst_d.ap()[s])
            fstt[s] = ft[:]
            f0 = state.tile([128, H], FP16, tag=f"f0_{s}")
            nc.sync.dma_start(f0[:], ft0_d.ap()[s])
            f0t[s] = f0[:]
            t = state.tile([128, H3], FP16, tag=f"S{s}")
            nc.gpsimd.memset(t[:, 0:H], 0.0)
            S[s] = t

        def setup_seq(s):
            ft = fstt[s]  # AP view [121,4,2,128]
            # Gram: pg[:, i, 0:256] = M[c1 in half i, :]
            pg = psmm.tile([128, 2, 512], F32, tag="mm")
            for i in range(2):
                for h in range(4):
                    nc.tensor.matmul(pg[:, i, 0:256], ft[:, h, i, :],
                                     ft[:, h, :, :], start=(h == 0), stop=(h == 3))
            mst = state.tile([128, 2, 2, 128], FP16, tag=f"Mst{s}")
            for i in range(2):
                nc.scalar.copy(mst[:, i, :, :], pg[:, i, 0:256].rearrange("p (t f) -> p t f", t=2))
            Mst[s] = mst
            # g0 = X(-T) + M f0: X(-T) first (independent of Mst evict)
            pf = psmm.tile([128, 2, 512], F32, tag="mm")
            for j in range(2):
                for h in range(4):
                    nc.tensor.matmul(pf[:, j, 0:F], ft[:, h, j, :], negT[h][:],
                                     start=(h == 0), stop=False)
                for i in range(2):
                    nc.tensor.matmul(pf[:, j, 0:F], mst[:, i, j, :],
                                     f0t[s][:, 484 * i:484 * i + 484],
                                     start=False, stop=(i == 1))
            nc.scalar.activation(S[s][:, H:H2].rearrange("p (t f) -> p t f", t=2),
                                 pf[:, :, 0:F], AF.Copy, scale=64.0)

        # alpha batching: seqs (0,1) share bank-pair A, (2,3) share B.
        pn, pd, alpha = {}, {}, {}

        fgMb_cur = {}

        def ph1_mm(i, s):
            pfgM = psmm.tile([128, 2, 512], F32, tag="mm")
            for j in range(2):
                for k in range(2):
                    nc.tensor.matmul(pfgM[:, j, 0:F], Mst[s][:, k, j, :],
                                     S[s][:, H + 484 * k:H + 484 * k + 484],
                                     start=(k == 0), stop=(k == 1))
            return pfgM

        def ph1_ew(i, s, pfgM, last):
            g = S[s][:, H:H2]
            nc.scalar.copy(S[s][:, H2:H3].rearrange("p (t f) -> p t f", t=2),
                           pfgM[:, :, 0:F])
            sq = work.tile([128, H], FP16, tag="sq")
            pool_sq = False
            nc.scalar.activation(sq[:], g, AF.Square, scale=sqs[:])
            td = work.tile([128, H], FP16, tag="td")
            nc.gpsimd.tensor_mul(td[:, 0:484], S[s][:, H2:H2 + 484], S[s][:, H:H + 484])
            nc.gpsimd.tensor_mul(td[:, 484:H], S[s][:, H2 + 484:H3], S[s][:, H + 484:H2])
            return sq, td, pool_sq

        def ph1_colsum(i, s, sq, td, pool_sq):
            pnd = psnd.tile([128, 2, 512], F32, tag="pnd")
            stat_n = onesps if pool_sq else onesp
            for j in range(2):
                nc.tensor.matmul(pnd[:, 0, 0:F], stat_n[:], sq[:, 484 * j:484 * j + 484],
                                 start=(j == 0), stop=(j == 1))
                nc.tensor.matmul(pnd[:, 1, 0:F], onesn[:], td[:, 484 * j:484 * j + 484],
                                 start=(j == 0), stop=(j == 1))
            rec = alph.tile([128, F], F32, tag="rec")
            act_recip(rec[:], pnd[:, 1, 0:F])
            ab = alph.tile([128, F], FP16, tag="ab")
            nc.vector.tensor_mul(ab[:], pnd[:, 0, 0:F], rec[:])
            alpha[s] = ab

        def ph2(i, s):
            last = i == num_iter - 1
            ab = alpha[s]
            if not last:
                ab4 = ab[:].unsqueeze(1).broadcast_to((128, 4, 484))
                uv = work.tile([128, H2], FP16, tag="uv")
                nc.vector.tensor_mul(
                    uv[:].rearrange("p (t f) -> p t f", t=4), ab4,
                    S[s][:, H:H3].rearrange("p (t f) -> p t f", t=4))
                nc.vector.tensor_add(S[s][:, 0:H2], S[s][:, 0:H2], uv[:])
            else:
                ab3 = ab[:].unsqueeze(1).broadcast_to((128, 2, 484))
                u = work.tile([128, H], FP16, tag="uv")
                nc.vector.tensor_mul(
                    u[:].rearrange("p (t f) -> p t f", t=2), ab3,
                    S[s][:, H:H2].rearrange("p (t f) -> p t f", t=2))
                nc.vector.tensor_add(S[s][:, 0:H], S[s][:, 0:H], u[:])
                fo = work.tile([128, H], F32, tag="fo")
                nc.vector.scalar_tensor_tensor(
                    fo[:], S[s][:, 0:H], 1.0 / 64.0, f0t[s][:],
                    ALU.mult, ALU.add)
                nc.sync.dma_start(out_d.ap()[s], fo[:])

        def emit_stage(it, s):
            pf = ph1_mm(it, s)
            sq, td, psq = ph1_ew(it, s, pf, it == num_iter - 1)
            ph1_colsum(it, s, sq, td, psq)
            ph2(it, s)

        for s in range(spc):
            setup_seq(s)
        for i in range(num_iter):
            for s in range(spc):
                pf = ph1_mm(i, s)
                sq, td, psq = ph1_ew(i, s, pf, i == num_iter - 1)
                ph1_colsum(i, s, sq, td, psq)
            for s in range(spc):
                ph2(i, s)

    nc.compile()
    return nc


def make_g_factors():
    """negT[(y,x),(fy,fx)] = -G[y,fy]*G[x,fx]; gy[p,h,fy]=G[y(121h+p),fy],
    gx[p,fx] = -G[x(121h+p),fx] (x pattern repeats every 121*? -> x=(121h+p)%22
    independent of h since 121 % 22 = 11... must use per-h x too!)"""
    k = np.arange(22, dtype=np.float64)
    G = np.exp(-0.5 * (k[:, None] - k[None, :]) ** 2)
    hw = np.arange(484)
    y, x = hw // 22, hw % 22
    gy = G[y].reshape(4, 121, 22).transpose(1, 0, 2)
    gx = (-G[x] / 8.0).reshape(4, 121, 22).transpose(1, 0, 2)
    return gy, gx


def make_in_maps(filter, feat, log_step_length, filter_reg, ncores=NCORES, spc=SPC):
    BF = np.float16
    gy, gx = make_g_factors()
    lsl = np.ascontiguousarray(log_step_length, np.float32)
    f = np.asarray(filter, np.float32)[:, :, :, 0, 0]          # (S, F, C)
    x = np.asarray(feat, np.float32)[0].reshape(S_TOTAL, C, F) / 8.0
    # fst[p,h,j,c] = X[128j+c, 121h+p]
    fst_all = x.reshape(S_TOTAL, 2, 128, 4, 121).transpose(0, 4, 3, 1, 2)
    # ft0[p, 484j+f] = f0[c=128j+p, f]  (f0 = f.transpose -> (C,F))
    ft0_all = f.transpose(0, 2, 1).reshape(S_TOTAL, 2, 128, F).transpose(0, 2, 1, 3)
    ft0_all = ft0_all.reshape(S_TOTAL, 128, H)
    in_maps = []
    for c in range(ncores):
        sl = slice(c * spc, (c + 1) * spc)
        in_maps.append({
            "fst": np.ascontiguousarray(fst_all[sl]).astype(BF),
            "ft0": np.ascontiguousarray(ft0_all[sl]).astype(BF),
            "gy": gy.astype(BF),
            "gx": gx.astype(BF),
            "lsl": lsl,
        })
    return in_maps


def assemble_output(results, ncores=NCORES, spc=SPC):
    out = np.empty((S_TOTAL, F, C), np.float32)
    for c in range(ncores):
        r = np.asarray(results[c]["outT"], dtype=np.float32)  # [spc,128,968]
        # r[s,p,484j+f] = f5[c=128j+p, f] -> out[s, f, c]
        out[c * spc:(c + 1) * spc] = r.reshape(spc, 128, 2, F).transpose(0, 3, 2, 1).reshape(spc, F, C)
    return out[:, :, :, None, None]


_nc_cache = None

from contextlib import contextmanager


@contextmanager
def _neuron_devices_visible():
    import os

    if "jax" not in sys.modules and os.environ.get("JAX_PLATFORMS") in ("cpu",):
        del os.environ["JAX_PLATFORMS"]
    import jax

    devs = jax.devices()
    if len(devs) >= NCORES and devs[0].platform != "cpu":
        yield
        return
    plat = None
    for cand in ("axon", "neuron"):
        try:
            if len(jax.devices(cand)) >= NCORES:
                plat = cand
                break
        except Exception:
            continue
    if plat is None:
        yield
        return
    real = jax.devices

    def patched(backend=None):
        return real(plat if backend is None else backend)

    jax.devices = patched
    try:
        yield
    finally:
        jax.devices = real


def kernel(filter, feat, test_anno, log_step_length, filter_reg):
    global _nc_cache
    if _nc_cache is None:
        _nc_cache = build()
    from concourse.bass_utils import run_bass_kernel_spmd

    in_maps = make_in_maps(filter, feat, log_step_length, filter_reg)
    with _neuron_devices_visible():
        res = run_bass_kernel_spmd(_nc_cache, in_maps, core_ids=list(range(NCORES)))
    return assemble_output(res.results)
